# revision 1
# baseline (speedup 1.0000x reference)
"""Trainium2 Bass kernel for DistributedAFNO2D.

Problem: x(2,768,256,256) f32; per-block (8 blocks of 96 ch) spectral MLP:
  out = irfft2( softshrink( W2*relu(W1*rfft2(x) + b1) + b2 ) ) + x
Block-diagonal channel mixing with shared-per-(u,v) complex 96x96 weights.

Sharding: block k -> core k (8 cores). No collectives. Each core handles
(2, 96, 256, 256) with its own block weights.

All FFTs are dense matmuls with probed DFT matrices (bf16 inputs, fp32 PSUM).
Dataflow per core, per batch b:
  Phase A (per channel c):
    S1 contract h: psY[w_chunk, (Yr-u256 | Yi-u256)] = x[h,w].T-matmul CHpack
    S2 contract w: psZ[u_chunk, (Zr-v129 | Zi-v129)] via R1/R2 consts
    -> Zbuf[b, u, c, 258] bf16 in DRAM
  Phase B (per pair of u):
    Z1 tile [98, 2, 258] (rows 96/97 = bias ones-pattern)
    mix1 (3 matmuls: W1R_aug*Z1 + split-sign W1I on swapped halves) + b1 (K-aug)
    relu (ACT) -> o1P [98, 2, 258] (rows 96/97 ones-pattern)
    mix2 (3 matmuls) + b2 -> psum t
    softshrink: c=clamp(t,+-lam); s=t-c (DVE) -> Sbuf[b, c, u, 258] bf16
  Phase C (per channel c):
    Q^T (v 1..128 on partitions) = s-slices x CHIpack matmuls; combines (DVE)
    DC term q0 = (CHIr.sr0 - CHIi.si0)/16 (N=1 matmuls)
    out[h,w] = QrT.Gc + QiT.Gs (matmuls) + x + q0  (DVE stt)
"""
import os
import sys
import numpy as np

sys.path.insert(0, "/opt/trn_rl_repo")

import ml_dtypes

BF16 = ml_dtypes.bfloat16

H = 256
W = 256
NV = W // 2 + 1  # 129
BLK = 96
NCORES = 8
B = 2
LAM = 0.01


def make_host_consts():
    """All packed constant matrices (numpy bf16) via probing np.fft."""
    I = np.eye(H, dtype=np.float64)
    F = np.fft.fft(I, axis=0, norm='ortho')       # F[u,h]; F@x = fft(x)
    Fi = np.fft.ifft(I, axis=0, norm='ortho')     # Fi[h,u]
    CHr = F.real.T.copy()                          # [h,u]
    CHi = F.imag.T.copy()
    EWr = F.real.T[:, :NV].copy()                  # [w,v]
    EWi = F.imag.T[:, :NV].copy()
    CHIr = Fi.real.T.copy()                        # [u,h]
    CHIi = Fi.imag.T.copy()
    Ir = np.eye(NV)
    Gc = np.fft.irfft(Ir, n=W, axis=-1, norm='ortho')        # [v,w]
    Gs = np.fft.irfft(1j * Ir, n=W, axis=-1, norm='ortho')   # [v,w]

    c = {}
    # CHpack [2][128, 512]: rows h (chunk), cols [CHr-u | CHi-u]
    c['chpack'] = np.stack([
        np.concatenate([CHr[j * 128:(j + 1) * 128, :], CHi[j * 128:(j + 1) * 128, :]], axis=1)
        for j in range(2)])
    # R1 [2][128, 258] = [EWr | EWi]; R2 = [-EWi | EWr] rows w chunk
    c['r1'] = np.stack([
        np.concatenate([EWr[j * 128:(j + 1) * 128], EWi[j * 128:(j + 1) * 128]], axis=1)
        for j in range(2)])
    c['r2'] = np.stack([
        np.concatenate([-EWi[j * 128:(j + 1) * 128], EWr[j * 128:(j + 1) * 128]], axis=1)
        for j in range(2)])
    # CHIpack [2][128, 512]: rows u chunk, cols [CHIr-h | CHIi-h]
    c['chipack'] = np.stack([
        np.concatenate([CHIr[j * 128:(j + 1) * 128], CHIi[j * 128:(j + 1) * 128]], axis=1)
        for j in range(2)])
    # NCHI [2][128, 256] = -CHIi rows u chunk
    c['nchi'] = np.stack([-CHIi[j * 128:(j + 1) * 128] for j in range(2)])
    # G tiles rows v=1..128
    c['gc'] = Gc[1:129]
    c['gs'] = Gs[1:129]
    return {k: v.astype(BF16) for k, v in c.items()}


def make_weight_consts(w1k, b1k, w2k, b2k):
    """Augmented weight matrices for one block.
    w1k/w2k: (96, 96, 2) [i, o, ri]; b1k/b2k: (96, 2) [o, ri]."""
    return {
        'w1r': w1k[..., 0].astype(BF16),
        'w1i': w1k[..., 1].astype(BF16),
        'w1in': (-w1k[..., 1]).astype(BF16),
        'w2r': w2k[..., 0].astype(BF16),
        'w2i': w2k[..., 1].astype(BF16),
        'w2in': (-w2k[..., 1]).astype(BF16),
    }


def build_nc():
    import concourse.bass as bass
    import concourse.tile as tile
    from concourse import bacc, mybir

    dt = mybir.dt
    nc = bacc.Bacc("TRN2", target_bir_lowering=False, debug=False)

    # I/O
    x32 = nc.dram_tensor("x32", [B, BLK, H, W], dt.float32, kind="ExternalInput").ap()
    xbf = nc.dram_tensor("xbf", [B, BLK, H, W], dt.bfloat16, kind="ExternalInput").ap()
    chpack = nc.dram_tensor("chpack", [2, 128, 512], dt.bfloat16, kind="ExternalInput").ap()
    r1 = nc.dram_tensor("r1", [2, 128, 258], dt.bfloat16, kind="ExternalInput").ap()
    r2 = nc.dram_tensor("r2", [2, 128, 258], dt.bfloat16, kind="ExternalInput").ap()
    chipack = nc.dram_tensor("chipack", [2, 128, 512], dt.bfloat16, kind="ExternalInput").ap()
    nchi = nc.dram_tensor("nchi", [2, 128, 256], dt.bfloat16, kind="ExternalInput").ap()
    gc = nc.dram_tensor("gc", [128, 256], dt.bfloat16, kind="ExternalInput").ap()
    gs = nc.dram_tensor("gs", [128, 256], dt.bfloat16, kind="ExternalInput").ap()
    wts = {n: nc.dram_tensor(n, [96, 96], dt.bfloat16, kind="ExternalInput").ap()
           for n in ['w1r', 'w1i', 'w1in', 'w2r', 'w2i', 'w2in']}
    b1cols = nc.dram_tensor("b1cols", [96, 2], dt.float32, kind="ExternalInput").ap()
    b2cols = nc.dram_tensor("b2cols", [96, 4], dt.float32, kind="ExternalInput").ap()
    out = nc.dram_tensor("out", [B, BLK, H, W], dt.float32, kind="ExternalOutput").ap()

    # DRAM scratch
    zbuf = nc.dram_tensor("zbuf", [B, H, BLK, 258], dt.bfloat16).ap()
    sbuf_d = nc.dram_tensor("sbufd", [B, BLK, H, 258], dt.bfloat16).ap()


    with tile.TileContext(nc) as tc:
        from contextlib import ExitStack
        with ExitStack() as ctx:
            consts = ctx.enter_context(tc.tile_pool(name="consts", bufs=1))
            pa_x = ctx.enter_context(tc.tile_pool(name="pa_x", bufs=4))
            pa_y = ctx.enter_context(tc.tile_pool(name="pa_y", bufs=4))
            pa_z = ctx.enter_context(tc.tile_pool(name="pa_z", bufs=4))
            pb_s = ctx.enter_context(tc.tile_pool(name="pb_s", bufs=4))
            pc_in = ctx.enter_context(tc.tile_pool(name="pc_in", bufs=4))
            pc_q = ctx.enter_context(tc.tile_pool(name="pc_q", bufs=4))
            pc_o = ctx.enter_context(tc.tile_pool(name="pc_o", bufs=4))
            # Single PSUM pool: 3 shared tags x (3+3+2) bufs x 1 bank = 8 banks
            psum = ctx.enter_context(tc.tile_pool(name="psum", bufs=1, space="PSUM"))

            # ---- Load constants (one [128, X] tile per chunk) ----
            def chunked_const(name, ap_, ncols):
                ts = []
                for j in range(2):
                    t = consts.tile([128, ncols], dt.bfloat16, tag=f"{name}{j}", name=f"{name}{j}")
                    nc.sync.dma_start(out=t, in_=ap_[j])
                    ts.append(t)
                return ts

            t_ch = chunked_const("t_ch", chpack, 512)
            t_r1 = chunked_const("t_r1", r1, 258)
            t_r2 = chunked_const("t_r2", r2, 258)
            t_chi = chunked_const("t_chi", chipack, 512)
            t_nchi = chunked_const("t_nchi", nchi, 256)
            t_gc = consts.tile([128, 256], dt.bfloat16, tag="t_gc", name="t_gc")
            nc.sync.dma_start(out=t_gc, in_=gc)
            t_gs = consts.tile([128, 256], dt.bfloat16, tag="t_gs", name="t_gs")
            nc.sync.dma_start(out=t_gs, in_=gs)
            t_w = {}
            for n, ap_ in wts.items():
                t_w[n] = consts.tile([96, 96], dt.bfloat16, tag=f"t_{n}", name=f"t_{n}")
                nc.sync.dma_start(out=t_w[n], in_=ap_)

            t_b2 = consts.tile([96, 4], dt.float32, tag="t_b2", name="t_b2")
            nc.sync.dma_start(out=t_b2, in_=b2cols)
            t_b1 = consts.tile([96, 2], dt.float32, tag="t_b1", name="t_b1")
            nc.sync.dma_start(out=t_b1, in_=b1cols)

            for b in range(B):
                # ================= Phase A =================
                for c in range(BLK):
                    xt0 = pa_x.tile([128, 256], dt.bfloat16, tag="xt0", name="xt0")
                    nc.sync.dma_start(out=xt0, in_=xbf[b, c, 0:128, :])
                    xt1 = pa_x.tile([128, 256], dt.bfloat16, tag="xt1", name="xt1")
                    nc.sync.dma_start(out=xt1, in_=xbf[b, c, 128:256, :])

                    ys = []
                    for wc in range(2):
                        psy = psum.tile([128, 512], dt.float32, tag="psA", name="psy", bufs=3)
                        nc.tensor.matmul(psy, lhsT=xt0[:, wc * 128:(wc + 1) * 128],
                                         rhs=t_ch[0], start=True, stop=False)
                        nc.tensor.matmul(psy, lhsT=xt1[:, wc * 128:(wc + 1) * 128],
                                         rhs=t_ch[1], start=False, stop=True)
                        y = pa_y.tile([128, 512], dt.bfloat16, tag=f"y{wc}", name=f"y{wc}")
                        nc.scalar.copy(y, psy)
                        ys.append(y)

                    for uc in range(2):
                        psz = psum.tile([128, 512], dt.float32, tag="psB", name="psz", bufs=3)
                        us = slice(uc * 128, (uc + 1) * 128)
                        us2 = slice(256 + uc * 128, 256 + (uc + 1) * 128)
                        nc.tensor.matmul(psz[:, 0:258], lhsT=ys[0][:, us], rhs=t_r1[0], start=True, stop=False)
                        nc.tensor.matmul(psz[:, 0:258], lhsT=ys[0][:, us2], rhs=t_r2[0], start=False, stop=False)
                        nc.tensor.matmul(psz[:, 0:258], lhsT=ys[1][:, us], rhs=t_r1[1], start=False, stop=False)
                        nc.tensor.matmul(psz[:, 0:258], lhsT=ys[1][:, us2], rhs=t_r2[1], start=False, stop=True)
                        zt = pa_z.tile([128, 258], dt.bfloat16, tag="zt", name="zt")
                        nc.scalar.copy(zt, psz[:, 0:258])
                        nc.sync.dma_start(out=zbuf[b, uc * 128:(uc + 1) * 128, c, :], in_=zt)

                # ================= Phase B =================
                for u in range(H):
                    z1 = pb_s.tile([96, 258], dt.bfloat16, tag="z1", name="z1")
                    nc.gpsimd.dma_start(out=z1, in_=zbuf[b, u, :, :])

                    ps1 = psum.tile([96, 512], dt.float32, tag="psA", name="ps1", bufs=3)
                    nc.tensor.matmul(ps1[:, 0:258], lhsT=t_w['w1r'], rhs=z1, start=True, stop=False,
                                     skip_group_check=True)
                    nc.tensor.matmul(ps1[:, 0:129], lhsT=t_w['w1in'], rhs=z1[:, 129:258],
                                     start=False, stop=True, skip_group_check=True)
                    nc.tensor.matmul(ps1[:, 129:258], lhsT=t_w['w1i'], rhs=z1[:, 0:129],
                                     start=False, stop=True, skip_group_check=True)

                    o1 = pb_s.tile([96, 258], dt.bfloat16, tag="o1", name="o1")
                    nc.scalar.activation(o1[:, 0:129], ps1[:, 0:129],
                                         mybir.ActivationFunctionType.Relu, bias=t_b1[:, 0:1])
                    nc.scalar.activation(o1[:, 129:258], ps1[:, 129:258],
                                         mybir.ActivationFunctionType.Relu, bias=t_b1[:, 1:2])

                    ps2 = psum.tile([96, 512], dt.float32, tag="psB", name="ps2", bufs=3)
                    nc.tensor.matmul(ps2[:, 0:258], lhsT=t_w['w2r'], rhs=o1, start=True, stop=False,
                                     skip_group_check=True)
                    nc.tensor.matmul(ps2[:, 0:129], lhsT=t_w['w2in'], rhs=o1[:, 129:258],
                                     start=False, stop=True, skip_group_check=True)
                    nc.tensor.matmul(ps2[:, 129:258], lhsT=t_w['w2i'], rhs=o1[:, 0:129],
                                     start=False, stop=True, skip_group_check=True)

                    # softshrink with b2 folded into clamp bounds:
                    # s = o2 - clamp(o2, -lam-b2, lam-b2)
                    cl = pb_s.tile([96, 258], dt.float32, tag="cl", name="cl")
                    nc.vector.tensor_scalar(cl[:, 0:129], ps2[:, 0:129], t_b2[:, 0:1], t_b2[:, 1:2],
                                            mybir.AluOpType.min, mybir.AluOpType.max)
                    nc.vector.tensor_scalar(cl[:, 129:258], ps2[:, 129:258], t_b2[:, 2:3], t_b2[:, 3:4],
                                            mybir.AluOpType.min, mybir.AluOpType.max)
                    st = pb_s.tile([96, 258], dt.bfloat16, tag="st", name="st")
                    nc.vector.tensor_tensor(st, ps2[:, 0:258], cl, mybir.AluOpType.subtract)
                    nc.sync.dma_start(out=sbuf_d[b, :, u, :], in_=st)

                # ================= Phase C =================
                for c in range(BLK):
                    st0 = pc_in.tile([128, 258], dt.bfloat16, tag="st0", name="st0")
                    nc.gpsimd.dma_start(out=st0, in_=sbuf_d[b, c, 0:128, :])
                    st1 = pc_in.tile([128, 258], dt.bfloat16, tag="st1", name="st1")
                    nc.gpsimd.dma_start(out=st1, in_=sbuf_d[b, c, 128:256, :])

                    # QrT = sr.CHIr - si.CHIi ; QiT = sr.CHIi + si.CHIr  (psum accum)
                    psa = psum.tile([128, 256], dt.float32, tag="psA", name="psa", bufs=3)
                    nc.tensor.matmul(psa, lhsT=st0[:, 1:129], rhs=t_chi[0][:, 0:256], start=True, stop=False)
                    nc.tensor.matmul(psa, lhsT=st1[:, 1:129], rhs=t_chi[1][:, 0:256], start=False, stop=False)
                    nc.tensor.matmul(psa, lhsT=st0[:, 130:258], rhs=t_nchi[0], start=False, stop=False)
                    nc.tensor.matmul(psa, lhsT=st1[:, 130:258], rhs=t_nchi[1], start=False, stop=True)
                    psb = psum.tile([128, 256], dt.float32, tag="psB", name="psb", bufs=3)
                    nc.tensor.matmul(psb, lhsT=st0[:, 1:129], rhs=t_chi[0][:, 256:512], start=True, stop=False)
                    nc.tensor.matmul(psb, lhsT=st1[:, 1:129], rhs=t_chi[1][:, 256:512], start=False, stop=False)
                    nc.tensor.matmul(psb, lhsT=st0[:, 130:258], rhs=t_chi[0][:, 0:256], start=False, stop=False)
                    nc.tensor.matmul(psb, lhsT=st1[:, 130:258], rhs=t_chi[1][:, 0:256], start=False, stop=True)

                    qr = pc_q.tile([128, 256], dt.bfloat16, tag="qr", name="qr")
                    nc.scalar.copy(qr, psa)
                    qi = pc_q.tile([128, 256], dt.bfloat16, tag="qi", name="qi")
                    nc.scalar.copy(qi, psb)

                    # DC (v=0) term -> q0 per h-chunk
                    psq = psum.tile([128, 2], dt.float32, tag="psC", name="psq", bufs=2)
                    for hc in range(2):
                        hs = slice(hc * 128, (hc + 1) * 128)
                        nc.tensor.matmul(psq[:, hc:hc + 1], lhsT=t_chi[0][:, hs], rhs=st0[:, 0:1],
                                         start=(hc == 0), stop=False, skip_group_check=True)
                        nc.tensor.matmul(psq[:, hc:hc + 1], lhsT=t_nchi[0][:, hs], rhs=st0[:, 129:130],
                                         start=False, stop=False, skip_group_check=True)
                        nc.tensor.matmul(psq[:, hc:hc + 1], lhsT=t_chi[1][:, hs], rhs=st1[:, 0:1],
                                         start=False, stop=False, skip_group_check=True)
                        nc.tensor.matmul(psq[:, hc:hc + 1], lhsT=t_nchi[1][:, hs], rhs=st1[:, 129:130],
                                         start=False, stop=True, skip_group_check=True)
                    q0 = pc_q.tile([128, 2], dt.float32, tag="q0", name="q0")
                    nc.vector.tensor_scalar_mul(q0, psq, 1.0 / 16.0)

                    for hc in range(2):
                        hs = slice(hc * 128, (hc + 1) * 128)
                        pso = psum.tile([128, 512], dt.float32, tag="psC", name="pso", bufs=2)[:, 0:256]
                        nc.tensor.matmul(pso, lhsT=qr[:, hs], rhs=t_gc, start=True, stop=False)
                        nc.tensor.matmul(pso, lhsT=qi[:, hs], rhs=t_gs, start=False, stop=True)
                        xt = pc_o.tile([128, 256], dt.float32, tag="xt", name="xt")
                        nc.sync.dma_start(out=xt, in_=x32[b, c, hs, :])
                        ot = pc_o.tile([128, 256], dt.float32, tag="ot", name="ot")
                        nc.vector.scalar_tensor_tensor(
                            ot, xt, q0[:, hc:hc + 1], pso,
                            mybir.AluOpType.add, mybir.AluOpType.add)
                        nc.sync.dma_start(out=out[b, c, hs, :], in_=ot)
    nc.compile()
    return nc


_NC_CACHE = {}


def _get_nc():
    if 'nc' not in _NC_CACHE:
        _NC_CACHE['nc'] = build_nc()
    return _NC_CACHE['nc']


def make_in_maps(x, w1, b1, w2, b2):
    hc = make_host_consts()
    x = np.ascontiguousarray(x, dtype=np.float32)
    in_maps = []
    for k in range(NCORES):
        xk = np.ascontiguousarray(x[:, BLK * k:BLK * (k + 1)])
        wk = make_weight_consts(w1[k], b1[k, :, 0, 0, :], w2[k], b2[k, :, 0, 0, :])
        b2k = b2[k, :, 0, 0, :]
        b2cols = np.stack([LAM - b2k[:, 0], -LAM - b2k[:, 0],
                           LAM - b2k[:, 1], -LAM - b2k[:, 1]], axis=1).astype(np.float32)
        b1cols = np.ascontiguousarray(b1[k, :, 0, 0, :], dtype=np.float32)
        m = dict(
            b1cols=b1cols,
            b2cols=b2cols,
            x32=xk,
            xbf=xk.astype(BF16),
            chpack=hc['chpack'], r1=hc['r1'], r2=hc['r2'],
            chipack=hc['chipack'], nchi=hc['nchi'], gc=hc['gc'], gs=hc['gs'],
            **wk,
        )
        in_maps.append(m)
    return in_maps


def kernel(x, w1, b1, w2, b2):
    from concourse.bass_utils import run_bass_kernel_spmd
    nc = _get_nc()
    in_maps = make_in_maps(np.asarray(x), np.asarray(w1), np.asarray(b1),
                           np.asarray(w2), np.asarray(b2))
    res = run_bass_kernel_spmd(nc, in_maps, core_ids=list(range(NCORES)))
    outs = [res.results[k]['out'] for k in range(NCORES)]
    return np.concatenate(outs, axis=1)



# revision 6
# speedup vs baseline: 1.4646x; 1.4646x over previous
"""Trainium2 Bass kernel for DistributedAFNO2D (v2).

out = irfft2( softshrink( W2*relu(W1*rfft2(x) + b1) + b2 ) ) + x
Block k -> core k; per core (2, 96, 256, 256), dense DFT matmuls.

v2 changes vs baseline:
- Phase A exploits Hermitian symmetry of the h-DFT of real x: S1 computes only
  u=0..128 (free 258 vs 512); the u=129..255 half of S2 is obtained from the
  mirrored/conjugated Y columns via R1/-R2 (rows stored un-reversed; phase C
  uses mirrored inverse-DFT constants for that chunk).
- Phase B processes 4 u-rows at a time with r/i-split [96, 512] tiles
  (4 big matmuls per layer instead of 12 small), relu/clamp/sub spread across
  scalar/vector/gpsimd engines. v=128 (Nyquist) and the v=0 DC inverse column
  are handled in batched side-paths using PE transposes (no DRAM scatter).
- DC term of the inverse (v=0) computed for all channels at once (8 matmuls
  per batch instead of 8 per channel).
- bf16 residual + bf16 output (halves residual/store DMA traffic).
- DMA spread across sync(loads)/gpsimd+scalar(stores) queues, merged patterns.
"""
import sys
import numpy as np

sys.path.insert(0, "/opt/trn_rl_repo")

import ml_dtypes

BF16 = ml_dtypes.bfloat16

H = 256
W = 256
NV = 129
BLK = 96
NCORES = 8
B = 2
LAM = 0.01
OUT_KEY = "outbf"


def make_host_consts():
    I = np.eye(H)
    F = np.fft.fft(I, axis=0, norm='ortho')       # F[u,h]
    Fi = np.fft.ifft(I, axis=0, norm='ortho')     # Fi[h,u]
    CHr = F.real.T.copy()                          # [h,u]
    CHi = F.imag.T.copy()
    EWr = F.real.T[:, :NV].copy()                  # [w,v]
    EWi = F.imag.T[:, :NV].copy()
    CHIr = Fi.real.T.copy()                        # [u,h]
    CHIi = Fi.imag.T.copy()
    Ir = np.eye(NV)
    Gc = np.fft.irfft(Ir, n=W, axis=-1, norm='ortho')        # [v,w]
    Gs = np.fft.irfft(1j * Ir, n=W, axis=-1, norm='ortho')   # [v,w]

    c = {}
    c['chS'] = np.stack([
        np.concatenate([CHr[j*128:(j+1)*128, :NV], CHi[j*128:(j+1)*128, :NV]], axis=1)
        for j in range(2)])                        # [hc][128, 258]
    c['r1'] = np.stack([
        np.concatenate([EWr[j*128:(j+1)*128], EWi[j*128:(j+1)*128]], axis=1)
        for j in range(2)])                        # [wc][128, 258]
    r2 = np.stack([
        np.concatenate([-EWi[j*128:(j+1)*128], EWr[j*128:(j+1)*128]], axis=1)
        for j in range(2)])
    c['r2'] = r2
    c['r2n'] = -r2
    m0 = np.arange(128)
    m1 = 255 - np.arange(128)                      # mirror rows for u-chunk 1
    c['chip'] = np.stack([
        np.concatenate([CHIr[m0], CHIi[m0]], axis=1),
        np.concatenate([CHIr[m1], CHIi[m1]], axis=1)])   # [uc][128, 512]
    c['nchi'] = np.stack([-CHIi[m0], -CHIi[m1]])   # [uc][128, 256]
    c['gc'] = Gc[1:129]                            # [128, 256]
    c['gs'] = Gs[1:129]
    c['ident'] = np.eye(128)
    return {k: v.astype(BF16) for k, v in c.items()}


def build_nc():
    import concourse.bass as bass
    import concourse.tile as tile
    from concourse import bacc, mybir

    dt = mybir.dt
    nc = bacc.Bacc("TRN2", target_bir_lowering=False, debug=False)

    xbf = nc.dram_tensor("xbf", [B, BLK, H, W], dt.bfloat16, kind="ExternalInput").ap()
    chS = nc.dram_tensor("chS", [2, 128, 258], dt.bfloat16, kind="ExternalInput").ap()
    r1 = nc.dram_tensor("r1", [2, 128, 258], dt.bfloat16, kind="ExternalInput").ap()
    r2 = nc.dram_tensor("r2", [2, 128, 258], dt.bfloat16, kind="ExternalInput").ap()
    r2n = nc.dram_tensor("r2n", [2, 128, 258], dt.bfloat16, kind="ExternalInput").ap()
    chip = nc.dram_tensor("chip", [2, 128, 512], dt.bfloat16, kind="ExternalInput").ap()
    nchi = nc.dram_tensor("nchi", [2, 128, 256], dt.bfloat16, kind="ExternalInput").ap()
    gc = nc.dram_tensor("gc", [128, 256], dt.bfloat16, kind="ExternalInput").ap()
    gs = nc.dram_tensor("gs", [128, 256], dt.bfloat16, kind="ExternalInput").ap()
    ident = nc.dram_tensor("ident", [128, 128], dt.bfloat16, kind="ExternalInput").ap()
    wts = {n: nc.dram_tensor(n, [96, 96], dt.bfloat16, kind="ExternalInput").ap()
           for n in ['w1r', 'w1i', 'w1in', 'w2r', 'w2i', 'w2in']}
    b1cols = nc.dram_tensor("b1cols", [96, 2], dt.float32, kind="ExternalInput").ap()
    b2cols = nc.dram_tensor("b2cols", [96, 4], dt.float32, kind="ExternalInput").ap()
    outbf = nc.dram_tensor("outbf", [B, BLK, H, W], dt.bfloat16, kind="ExternalOutput").ap()

    # DRAM scratch
    zbuf = nc.dram_tensor("zbuf", [B, H, BLK, 256], dt.bfloat16).ap()      # [b,row,c,(ri,v128)]
    sbufd = nc.dram_tensor("sbufd", [B, BLK, H, 258], dt.bfloat16).ap()    # [b,c,row,(ri,v129)]

    RELU = None  # set below

    with tile.TileContext(nc) as tc:
        from contextlib import ExitStack
        with ExitStack() as ctx:
            consts = ctx.enter_context(tc.tile_pool(name="consts", bufs=1))
            pa = ctx.enter_context(tc.tile_pool(name="pa", bufs=3))
            pb = ctx.enter_context(tc.tile_pool(name="pb", bufs=3))
            pc = ctx.enter_context(tc.tile_pool(name="pc", bufs=3))
            pst = ctx.enter_context(tc.tile_pool(name="pst", bufs=2))  # persistent per-batch
            psum = ctx.enter_context(tc.tile_pool(name="psum", bufs=1, space="PSUM"))

            RELU = mybir.ActivationFunctionType.Relu
            ADD = mybir.AluOpType.add
            MAXOP = mybir.AluOpType.max
            MINOP = mybir.AluOpType.min
            SUB = mybir.AluOpType.subtract

            def split_hc(ap2d, n=2):
                # [ (n h) w ] -> [h, n, w] permuted AP
                return ap2d.rearrange("(n h) w -> n h w", n=n).transpose([1, 0, 2])

            def cload(name, ap_, shape):
                t = consts.tile(list(shape), dt.bfloat16, tag=name, name=name)
                nc.sync.dma_start(out=t, in_=ap_)
                return t

            t_chS = [cload(f"chS{j}", chS[j], [128, 258]) for j in range(2)]
            t_r1 = [cload(f"r1{j}", r1[j], [128, 258]) for j in range(2)]
            t_r2 = [cload(f"r2{j}", r2[j], [128, 258]) for j in range(2)]
            t_r2n = [cload(f"r2n{j}", r2n[j], [128, 258]) for j in range(2)]
            t_chip = [cload(f"chip{j}", chip[j], [128, 512]) for j in range(2)]
            t_nchi = [cload(f"nchi{j}", nchi[j], [128, 256]) for j in range(2)]
            t_gc = cload("gc", gc, [128, 256])
            t_gs = cload("gs", gs, [128, 256])
            t_id = cload("ident", ident, [128, 128])
            t_w = {n: cload(n, ap_, [96, 96]) for n, ap_ in wts.items()}
            t_b1 = consts.tile([96, 2], dt.float32, tag="b1", name="t_b1")
            nc.sync.dma_start(out=t_b1, in_=b1cols)
            t_b2 = consts.tile([96, 4], dt.float32, tag="b2", name="t_b2")
            nc.sync.dma_start(out=t_b2, in_=b2cols)

            for b in range(B):
                # per-batch persistent SBUF tiles
                znyqA = [pst.tile([128, 192], dt.bfloat16, tag=f"znyqA{uc}", name=f"znyqA{uc}")
                         for uc in range(2)]       # [p, (ri,c)]
                zN = pst.tile([96, 512], dt.bfloat16, tag="zN", name="zN")   # [c, (ri, row256)]
                s0acc = pst.tile([96, 512], dt.bfloat16, tag="s0acc", name="s0acc")  # [c,(ri,row)]
                snyqT = [pst.tile([128, 192], dt.bfloat16, tag=f"snyqT{uc}", name=f"snyqT{uc}")
                         for uc in range(2)]
                q0sb = [pst.tile([128, 96], dt.float32, tag=f"q0sb{hc}", name=f"q0sb{hc}")
                        for hc in range(2)]

                # ================= Phase A =================
                for c in range(BLK):
                    xt = pa.tile([128, 512], dt.bfloat16, tag="xt", name="xt")
                    nc.sync.dma_start(
                        out=xt, in_=split_hc(xbf[b, c]))
                    ys = []
                    for wc in range(2):
                        psy = psum.tile([128, 258], dt.float32, tag="pgA", name="psy", bufs=4)
                        nc.tensor.matmul(psy, lhsT=xt[:, wc*128:wc*128+128],
                                         rhs=t_chS[0], start=True, stop=False)
                        nc.tensor.matmul(psy, lhsT=xt[:, 256+wc*128:256+wc*128+128],
                                         rhs=t_chS[1], start=False, stop=True)
                        y = pa.tile([128, 258], dt.bfloat16, tag=f"y{wc}", name=f"y{wc}")
                        nc.scalar.copy(y, psy)
                        ys.append(y)

                    ztm = pa.tile([128, 512], dt.bfloat16, tag="ztm", name="ztm")
                    for uc in range(2):
                        psz = psum.tile([128, 258], dt.float32, tag="pgA", name="psz", bufs=4)
                        if uc == 0:
                            sl_r, sl_i, rB = slice(0, 128), slice(129, 257), t_r2
                        else:
                            sl_r, sl_i, rB = slice(1, 129), slice(130, 258), t_r2n
                        nc.tensor.matmul(psz, lhsT=ys[0][:, sl_r], rhs=t_r1[0], start=True, stop=False)
                        nc.tensor.matmul(psz, lhsT=ys[0][:, sl_i], rhs=rB[0], start=False, stop=False)
                        nc.tensor.matmul(psz, lhsT=ys[1][:, sl_r], rhs=t_r1[1], start=False, stop=False)
                        nc.tensor.matmul(psz, lhsT=ys[1][:, sl_i], rhs=rB[1], start=False, stop=True)
                        pszv = psz.rearrange("p (ri v) -> p ri v", ri=2)
                        # wide part: v 0..127 -> ztm cols (uc, ri, v)
                        nc.vector.tensor_scalar_add(
                            ztm[:, uc*256:(uc+1)*256].rearrange("p (ri v) -> p ri v", ri=2),
                            pszv[:, :, 0:128], 0.0)
                        # nyquist col -> znyqA
                        nc.scalar.copy(
                            znyqA[uc].rearrange("p (ri c) -> p ri c", ri=2)[:, :, c],
                            pszv[:, :, 128])
                    nc.gpsimd.dma_start(out=split_hc(zbuf[b, :, c, :]), in_=ztm)

                # A-end: transpose znyqA -> zN [96 c, (ri, row)]
                for uc in range(2):
                    for ri in range(2):
                        tp = psum.tile([96, 128], dt.bfloat16, tag="pgB", name="tpz", bufs=4)
                        nc.tensor.transpose(tp, znyqA[uc][:, ri*96:(ri+1)*96], t_id)
                        nc.scalar.copy(
                            zN[:, ri*256+uc*128:ri*256+uc*128+128], tp)

                # ================= Phase B =================
                b1r, b1i = t_b1[:, 0:1], t_b1[:, 1:2]
                for g in range(64):
                    r0 = 4*g
                    zri = pb.tile([96, 1024], dt.bfloat16, tag="zri", name="zri")
                    zsrc = zbuf[b, r0:r0+4, :, :].rearrange("u c (ri v) -> u c ri v", ri=2)
                    for ri in range(2):
                        nc.sync.dma_start(
                            out=zri[:, ri*512:(ri+1)*512],
                            in_=zsrc[:, :, ri, :].transpose([1, 0, 2]))
                    zr, zi = zri[:, 0:512], zri[:, 512:1024]

                    ps1r = psum.tile([96, 512], dt.float32, tag="pgB", name="ps1r", bufs=4)
                    ps1i = psum.tile([96, 512], dt.float32, tag="pgB", name="ps1i", bufs=4)
                    nc.tensor.matmul(ps1r, lhsT=t_w['w1r'], rhs=zr, start=True, stop=False)
                    nc.tensor.matmul(ps1r, lhsT=t_w['w1in'], rhs=zi, start=False, stop=True)
                    nc.tensor.matmul(ps1i, lhsT=t_w['w1i'], rhs=zr, start=True, stop=False)
                    nc.tensor.matmul(ps1i, lhsT=t_w['w1r'], rhs=zi, start=False, stop=True)

                    o1 = pb.tile([96, 1024], dt.bfloat16, tag="o1", name="o1")
                    nc.scalar.activation(o1[:, 0:512], ps1r, RELU, bias=b1r)
                    nc.vector.tensor_scalar(o1[:, 512:1024], ps1i, b1i, 0.0, ADD, MAXOP)

                    ps2r = psum.tile([96, 512], dt.float32, tag="pgB", name="ps2r", bufs=4)
                    ps2i = psum.tile([96, 512], dt.float32, tag="pgB", name="ps2i", bufs=4)
                    nc.tensor.matmul(ps2r, lhsT=t_w['w2r'], rhs=o1[:, 0:512], start=True, stop=False)
                    nc.tensor.matmul(ps2r, lhsT=t_w['w2in'], rhs=o1[:, 512:1024], start=False, stop=True)
                    nc.tensor.matmul(ps2i, lhsT=t_w['w2i'], rhs=o1[:, 0:512], start=True, stop=False)
                    nc.tensor.matmul(ps2i, lhsT=t_w['w2r'], rhs=o1[:, 512:1024], start=False, stop=True)

                    tt = pb.tile([96, 1024], dt.bfloat16, tag="tt", name="tt")
                    nc.scalar.copy(tt[:, 0:512], ps2r)
                    nc.scalar.copy(tt[:, 512:1024], ps2i)
                    clr = pb.tile([96, 512], dt.bfloat16, tag="clr", name="clr")
                    cli = pb.tile([96, 512], dt.bfloat16, tag="cli", name="cli")
                    nc.gpsimd.tensor_scalar(clr, tt[:, 0:512], t_b2[:, 0:1], t_b2[:, 1:2], MINOP, MAXOP)
                    nc.gpsimd.tensor_scalar(cli, tt[:, 512:1024], t_b2[:, 2:3], t_b2[:, 3:4], MINOP, MAXOP)
                    stt = pb.tile([96, 1024], dt.bfloat16, tag="stt", name="stt")
                    nc.vector.tensor_tensor(stt[:, 0:512], tt[:, 0:512], clr, SUB)
                    nc.vector.tensor_tensor(stt[:, 512:1024], tt[:, 512:1024], cli, SUB)
                    # v=0 cols -> s0acc
                    sttv = stt.rearrange("p (ri u v) -> p ri u v", ri=2, u=4)
                    nc.scalar.copy(
                        s0acc.rearrange("p (ri r) -> p ri r", ri=2)[:, :, r0:r0+4],
                        sttv[:, :, :, 0])
                    sdst = sbufd[b, :, r0:r0+4, :].rearrange("c u (ri v) -> c u ri v", ri=2)
                    for ri in range(2):
                        nc.gpsimd.dma_start(
                            out=sdst[:, :, ri, 0:128],
                            in_=stt[:, ri*512:(ri+1)*512])

                # B-nyq: MLP on zN [96, (ri, 256)]
                zNr, zNi = zN[:, 0:256], zN[:, 256:512]
                psnr = psum.tile([96, 256], dt.float32, tag="pgB", name="psnr", bufs=4)
                psni = psum.tile([96, 256], dt.float32, tag="pgB", name="psni", bufs=4)
                nc.tensor.matmul(psnr, lhsT=t_w['w1r'], rhs=zNr, start=True, stop=False)
                nc.tensor.matmul(psnr, lhsT=t_w['w1in'], rhs=zNi, start=False, stop=True)
                nc.tensor.matmul(psni, lhsT=t_w['w1i'], rhs=zNr, start=True, stop=False)
                nc.tensor.matmul(psni, lhsT=t_w['w1r'], rhs=zNi, start=False, stop=True)
                o1n = pb.tile([96, 512], dt.bfloat16, tag="o1n", name="o1n")
                nc.scalar.activation(o1n[:, 0:256], psnr, RELU, bias=b1r)
                nc.vector.tensor_scalar(o1n[:, 256:512], psni, b1i, 0.0, ADD, MAXOP)
                psnr2 = psum.tile([96, 256], dt.float32, tag="pgB", name="psnr2", bufs=4)
                psni2 = psum.tile([96, 256], dt.float32, tag="pgB", name="psni2", bufs=4)
                nc.tensor.matmul(psnr2, lhsT=t_w['w2r'], rhs=o1n[:, 0:256], start=True, stop=False)
                nc.tensor.matmul(psnr2, lhsT=t_w['w2in'], rhs=o1n[:, 256:512], start=False, stop=True)
                nc.tensor.matmul(psni2, lhsT=t_w['w2i'], rhs=o1n[:, 0:256], start=True, stop=False)
                nc.tensor.matmul(psni2, lhsT=t_w['w2r'], rhs=o1n[:, 256:512], start=False, stop=True)
                ttn = pb.tile([96, 512], dt.bfloat16, tag="ttn", name="ttn")
                nc.scalar.copy(ttn[:, 0:256], psnr2)
                nc.scalar.copy(ttn[:, 256:512], psni2)
                clnr = pb.tile([96, 256], dt.bfloat16, tag="clnr", name="clnr")
                clni = pb.tile([96, 256], dt.bfloat16, tag="clni", name="clni")
                nc.gpsimd.tensor_scalar(clnr, ttn[:, 0:256], t_b2[:, 0:1], t_b2[:, 1:2], MINOP, MAXOP)
                nc.gpsimd.tensor_scalar(clni, ttn[:, 256:512], t_b2[:, 2:3], t_b2[:, 3:4], MINOP, MAXOP)
                sN = pb.tile([96, 512], dt.bfloat16, tag="sN", name="sN")
                nc.vector.tensor_tensor(sN[:, 0:256], ttn[:, 0:256], clnr, SUB)
                nc.vector.tensor_tensor(sN[:, 256:512], ttn[:, 256:512], clni, SUB)
                # transpose sN -> snyqT[uc] [128 p, (ri, c)]
                for uc in range(2):
                    for ri in range(2):
                        tp = psum.tile([128, 96], dt.bfloat16, tag="pgB", name="tps", bufs=4)
                        nc.tensor.transpose(tp, sN[:, ri*256+uc*128:ri*256+uc*128+128],
                                            t_id[0:96, 0:96])
                        nc.scalar.copy(snyqT[uc][:, ri*96:(ri+1)*96], tp)

                # DC batch: s0acc -> s0T -> q0sb
                s0T = [pst.tile([128, 192], dt.bfloat16, tag=f"s0T{uc}", name=f"s0T{uc}")
                       for uc in range(2)]
                for uc in range(2):
                    for ri in range(2):
                        tp = psum.tile([128, 96], dt.bfloat16, tag="pgB", name="tp0", bufs=4)
                        nc.tensor.transpose(tp, s0acc[:, ri*256+uc*128:ri*256+uc*128+128],
                                            t_id[0:96, 0:96])
                        nc.scalar.copy(s0T[uc][:, ri*96:(ri+1)*96], tp)
                for hc in range(2):
                    psq = psum.tile([128, 96], dt.float32, tag="pgB", name="psq", bufs=4)
                    for uc in range(2):
                        nc.tensor.matmul(psq, lhsT=t_chip[uc][:, hc*128:(hc+1)*128],
                                         rhs=s0T[uc][:, 0:96], start=(uc == 0), stop=False)
                        nc.tensor.matmul(psq, lhsT=t_nchi[uc][:, hc*128:(hc+1)*128],
                                         rhs=s0T[uc][:, 96:192], start=False, stop=(uc == 1))
                    nc.vector.tensor_scalar_mul(q0sb[hc], psq, 1.0/16.0)

                # ================= Phase C =================
                for c in range(BLK):
                    st = pc.tile([128, 516], dt.bfloat16, tag="st", name="st")
                    nc.sync.dma_start(
                        out=st,
                        in_=split_hc(sbufd[b, c]))
                    stv = st.rearrange("p (uc ri v) -> p uc ri v", uc=2, ri=2)
                    for uc in range(2):
                        nc.gpsimd.tensor_scalar_add(
                            stv[:, uc, :, 128],
                            snyqT[uc].rearrange("p (ri c) -> p ri c", ri=2)[:, :, c], 0.0)

                    psa = psum.tile([128, 256], dt.float32, tag="pgA", name="psa", bufs=4)
                    psb = psum.tile([128, 256], dt.float32, tag="pgA", name="psb", bufs=4)
                    # lhsT slices: uc0 r v1..128 = cols 1:129; uc0 i = 130:258
                    # uc1 r = 259:387; uc1 i = 388:516
                    nc.tensor.matmul(psa, lhsT=st[:, 1:129], rhs=t_chip[0][:, 0:256], start=True, stop=False)
                    nc.tensor.matmul(psa, lhsT=st[:, 130:258], rhs=t_nchi[0], start=False, stop=False)
                    nc.tensor.matmul(psa, lhsT=st[:, 259:387], rhs=t_chip[1][:, 0:256], start=False, stop=False)
                    nc.tensor.matmul(psa, lhsT=st[:, 388:516], rhs=t_nchi[1], start=False, stop=True)
                    nc.tensor.matmul(psb, lhsT=st[:, 1:129], rhs=t_chip[0][:, 256:512], start=True, stop=False)
                    nc.tensor.matmul(psb, lhsT=st[:, 130:258], rhs=t_chip[0][:, 0:256], start=False, stop=False)
                    nc.tensor.matmul(psb, lhsT=st[:, 259:387], rhs=t_chip[1][:, 256:512], start=False, stop=False)
                    nc.tensor.matmul(psb, lhsT=st[:, 388:516], rhs=t_chip[1][:, 0:256], start=False, stop=True)

                    qr = pc.tile([128, 256], dt.bfloat16, tag="qr", name="qr")
                    nc.scalar.copy(qr, psa)
                    qi = pc.tile([128, 256], dt.bfloat16, tag="qi", name="qi")
                    nc.vector.tensor_scalar_add(qi, psb, 0.0)

                    xres = pc.tile([128, 512], dt.bfloat16, tag="xres", name="xres")
                    nc.sync.dma_start(
                        out=xres, in_=split_hc(xbf[b, c]))
                    ot = pc.tile([128, 512], dt.bfloat16, tag="ot", name="ot")
                    for hc in range(2):
                        pso = psum.tile([128, 256], dt.float32, tag="pgB", name="pso", bufs=4)
                        nc.tensor.matmul(pso, lhsT=qr[:, hc*128:(hc+1)*128], rhs=t_gc, start=True, stop=False)
                        nc.tensor.matmul(pso, lhsT=qi[:, hc*128:(hc+1)*128], rhs=t_gs, start=False, stop=True)
                        nc.vector.scalar_tensor_tensor(
                            ot[:, hc*256:(hc+1)*256], xres[:, hc*256:(hc+1)*256],
                            q0sb[hc][:, c:c+1], pso, ADD, ADD)
                    nc.scalar.dma_start(out=split_hc(outbf[b, c]), in_=ot)
    nc.compile()
    return nc


_NC_CACHE = {}


def _get_nc():
    if 'nc' not in _NC_CACHE:
        _NC_CACHE['nc'] = build_nc()
    return _NC_CACHE['nc']


def make_in_maps(x, w1, b1, w2, b2):
    hc = make_host_consts()
    x = np.ascontiguousarray(x, dtype=np.float32)
    in_maps = []
    for k in range(NCORES):
        xk = np.ascontiguousarray(x[:, BLK*k:BLK*(k+1)])
        w1k, w2k = w1[k], w2[k]
        b1k = b1[k, :, 0, 0, :]
        b2k = b2[k, :, 0, 0, :]
        b2colsk = np.stack([LAM - b2k[:, 0], -LAM - b2k[:, 0],
                            LAM - b2k[:, 1], -LAM - b2k[:, 1]], axis=1).astype(np.float32)
        m = dict(
            xbf=xk.astype(BF16),
            chS=hc['chS'], r1=hc['r1'], r2=hc['r2'], r2n=hc['r2n'],
            chip=hc['chip'], nchi=hc['nchi'], gc=hc['gc'], gs=hc['gs'], ident=hc['ident'],
            w1r=w1k[..., 0].astype(BF16), w1i=w1k[..., 1].astype(BF16),
            w1in=(-w1k[..., 1]).astype(BF16),
            w2r=w2k[..., 0].astype(BF16), w2i=w2k[..., 1].astype(BF16),
            w2in=(-w2k[..., 1]).astype(BF16),
            b1cols=np.ascontiguousarray(b1k, dtype=np.float32),
            b2cols=b2colsk,
        )
        in_maps.append(m)
    return in_maps


def kernel(x, w1, b1, w2, b2):
    from concourse.bass_utils import run_bass_kernel_spmd
    nc = _get_nc()
    in_maps = make_in_maps(np.asarray(x), np.asarray(w1), np.asarray(b1),
                           np.asarray(w2), np.asarray(b2))
    res = run_bass_kernel_spmd(nc, in_maps, core_ids=list(range(NCORES)))
    outs = [np.asarray(res.results[k][OUT_KEY]).astype(np.float32) for k in range(NCORES)]
    return np.concatenate(outs, axis=1)
